# revision 1
# baseline (speedup 1.0000x reference)
import numpy as np
from contextlib import ExitStack

import concourse.bacc as bacc
import concourse.tile as tile
from concourse import mybir
from concourse.bass_utils import run_bass_kernel_spmd
from concourse.masks import make_identity

B, N, C, H, D = 2, 2048, 1024, 16, 64
BN = B * N
HL = H // 8
CL = HL * D
N_CORES = 8
NQC = 1024
NMT = N // 128

F32 = mybir.dt.float32
F32R = mybir.dt.float32r

USE_F32R = True
PROFILE = False
INTERLEAVE_B = True
DEFER_NORM = True

_CACHE = {}


def _enable_ldw_opt():
    import concourse.bass_utils as bu
    if getattr(bu, "_ldw_patched", False):
        return
    orig = bu.run_command

    def patched(argv, **kw):
        argv = ["--enable-ldw-opt=true" if a == "--enable-ldw-opt=false" else a
                for a in argv]
        return orig(argv, **kw)

    bu.run_command = patched
    bu._ldw_patched = True


def _mmdt():
    return F32R if USE_F32R else F32


def _build_nc():
    _enable_ldw_opt()
    nc = bacc.Bacc("TRN2", target_bir_lowering=False, debug=False,
                   num_devices=N_CORES)
    MMDT = _mmdt()
    x_d = nc.dram_tensor("x", [BN, C], MMDT, kind="ExternalInput")
    w_d = nc.dram_tensor("w", [C, 3 * CL], MMDT, kind="ExternalInput")
    wp_d = nc.dram_tensor("wp", [CL, C], MMDT, kind="ExternalInput")
    y_d = nc.dram_tensor("y", [BN, C], F32, kind="ExternalOutput")

    with tile.TileContext(nc) as tc:
        with ExitStack() as ctx:
            _emit(nc, tc, ctx, x_d, w_d, wp_d, y_d)
    nc.finalize()
    return nc


def _emit(nc, tc, ctx, x_d, w_d, wp_d, y_d):
    MMDT = _mmdt()
    const = ctx.enter_context(tc.tile_pool(name="const", bufs=1))

    ident_f32 = const.tile([128, 128], F32)
    make_identity(nc, ident_f32[:])
    if MMDT is F32:
        ident = ident_f32
    else:
        ident = const.tile([128, 128], MMDT)
        nc.vector.tensor_copy(ident[:], ident_f32[:])
    identB = const.tile([128, 64], MMDT)
    nc.sync.dma_start(identB[64:128, :], ident[0:64, 0:64])
    ones_t = const.tile([65, 64], F32)
    nc.gpsimd.memset(ones_t[64:65, :], 1.0)

    w_sb = const.tile([128, 8, 3 * CL], MMDT)
    nc.sync.dma_start(w_sb[:], w_d.ap().rearrange("(kt p) c -> p kt c", p=128))
    wp_sb = const.tile([64, HL, C], MMDT)
    nc.sync.dma_start(wp_sb[:], wp_d.ap().rearrange("(h p) c -> p h c", p=64))

    qkvT = []
    vaug = []
    outT = []
    for b in range(B):
        qkvT_b = const.tile([128, 3, N], MMDT, name=f"qkvT{b}")
        qkvT.append(qkvT_b)
        vaug_b = const.tile([128, HL, NMT, 65], MMDT, name=f"vaug{b}")
        vaug.append(vaug_b)
        outT_b = const.tile([64, HL, N], MMDT, name=f"outT{b}")
        outT.append(outT_b)
    ones_st = const.tile([128, HL * NMT], F32)
    nc.gpsimd.memset(ones_st[:], 1.0)
    for b in range(B):
        nc.vector.tensor_copy(
            vaug[b][:, :, :, 64:65],
            ones_st[:].rearrange("p (a b c) -> p a b c", a=HL, b=NMT, c=1),
        )

    bctx = ExitStack()
    xn_pool = bctx.enter_context(tc.tile_pool(name="xn", bufs=6))
    xt_pool = bctx.enter_context(tc.tile_pool(name="xt", bufs=16))
    ps_t = bctx.enter_context(tc.tile_pool(name="ps_t", bufs=2, space="PSUM"))
    ps_q = bctx.enter_context(tc.tile_pool(name="ps_q", bufs=2, space="PSUM"))

    def emit_chunk(nch):
        b, lc = nch // 4, nch % 4
        xns = []
        for t in range(4):
            xn = xn_pool.tile([128, C], MMDT, tag="xn")
            r0 = nch * 512 + t * 128
            nc.sync.dma_start(xn[:], x_d.ap()[r0:r0 + 128, :])
            xns.append(xn)
        xts = []
        for ct in range(8):
            pt = ps_t.tile([128, 512], MMDT, tag="pst")
            for t in range(4):
                nc.tensor.transpose(
                    pt[:, t * 128:(t + 1) * 128],
                    xns[t][:, ct * 128:(ct + 1) * 128],
                    ident[:],
                )
            xt = xt_pool.tile([128, 512], MMDT, tag="xt")
            nc.vector.tensor_copy(xt[:], pt[:])
            xts.append(xt)
        for co in range(3):
            pq = ps_q.tile([128, 512], F32, tag="psq")
            for ct in range(8):
                nc.tensor.matmul(
                    pq[:],
                    w_sb[:, ct, co * 128:(co + 1) * 128],
                    xts[ct][:],
                    start=(ct == 0), stop=(ct == 7),
                )
            nc.vector.tensor_copy(
                qkvT[b][:, co, lc * 512:(lc + 1) * 512], pq[:])
        pv = ps_t.tile([128, 512], MMDT, tag="pst")
        for h in range(HL):
            idn = ident if h == 0 else identB
            for ml in range(4):
                mt = lc * 4 + ml
                nc.tensor.transpose(
                    pv[:, (h * 4 + ml) * 64:(h * 4 + ml + 1) * 64],
                    qkvT[b][h * 64:(h + 1) * 64, 2,
                            mt * 128:(mt + 1) * 128],
                    idn[h * 64:(h + 1) * 64, 0:64],
                )
        nc.vector.tensor_copy(
            vaug[b][:, :, lc * 4:(lc + 1) * 4, 0:64],
            pv[:].rearrange("p (h m d) -> p h m d", h=HL, m=4),
        )

    s_pool = None

    def open_d_pools():
        nonlocal s_pool, o_pool, p_pool, n_pool, y_pool
        s_pool = ctx.enter_context(tc.tile_pool(name="ps_s", bufs=2, space="PSUM"))
        o_pool = ctx.enter_context(tc.tile_pool(name="ps_o", bufs=2, space="PSUM"))
        p_pool = ctx.enter_context(tc.tile_pool(name="pt", bufs=4))
        n_pool = ctx.enter_context(tc.tile_pool(name="nrm", bufs=2))
        y_pool = ctx.enter_context(tc.tile_pool(name="ysb", bufs=2))

    o_pool = p_pool = n_pool = y_pool = None
    NQC = 1024

    def emit_s_pair(b, q0, mt):
        tiles = []
        for h in range(HL):
            hs = slice(h * 64, (h + 1) * 64)
            ps_s = s_pool.tile([128, NQC], F32, tag="pss")
            for j in range(0, NQC, 512):
                nc.tensor.matmul(
                    ps_s[:, j:j + 512],
                    qkvT[b][hs, 1, mt * 128:(mt + 1) * 128],
                    qkvT[b][hs, 0, q0 + j:q0 + j + 512],
                    start=True, stop=True,
                )
            tiles.append(ps_s)
        return tiles

    def emit_normalize(b, q0, o_tiles):
        for h in range(HL):
            ps_o = o_tiles[h]
            rec = n_pool.tile([65, NQC], F32, tag="rec")
            nc.vector.reciprocal(rec[64:65, :], ps_o[64:65, :])
            ps_b = s_pool.tile([64, NQC], F32, tag="pss")
            for j in range(0, NQC, 512):
                nc.tensor.matmul(
                    ps_b[:, j:j + 512],
                    ones_t[64:65, :],
                    rec[64:65, j:j + 512],
                    start=True, stop=True,
                )
            rb = n_pool.tile([64, NQC], F32, tag="rb")
            nc.vector.tensor_copy(rb[:], ps_b[:])
            nc.vector.tensor_mul(
                outT[b][:, h, q0:q0 + NQC], ps_o[0:64, :], rb[:])

    def emit_proj(b, q0):
        for ln in range(q0 // 128, (q0 + NQC) // 128):
            nt = b * (N // 128) + ln
            y_sb = y_pool.tile([128, C], F32, tag="ysb")
            for j in range(0, C, 512):
                ps_y = s_pool.tile([128, 512], F32, tag="pss")
                for h in range(HL):
                    nc.tensor.matmul(
                        ps_y[:],
                        outT[b][:, h, ln * 128:(ln + 1) * 128],
                        wp_sb[:, h, j:j + 512],
                        start=(h == 0), stop=(h == HL - 1),
                    )
                nc.vector.tensor_copy(y_sb[:, j:j + 512], ps_y[:])
            nc.sync.dma_start(y_d.ap()[nt * 128:(nt + 1) * 128, :], y_sb[:])

    for nch in range(8):
        emit_chunk(nch)
    bctx.close()
    open_d_pools()
    pending = None
    for b in range(B):
        for q0 in range(0, N, NQC):
            o_tiles = []
            for h in range(HL):
                ps_o = o_pool.tile([65, NQC], F32, tag="pso")
                o_tiles.append(ps_o)
            s_tiles = emit_s_pair(b, q0, 0)
            for mt in range(NMT):
                p_tiles = []
                for h in range(HL):
                    pT = p_pool.tile([128, NQC], MMDT, tag="pT")
                    nc.scalar.activation(
                        pT[:], s_tiles[h][:],
                        mybir.ActivationFunctionType.Exp)
                    p_tiles.append(pT)
                if mt + 1 < NMT:
                    s_tiles = emit_s_pair(b, q0, mt + 1)
                if DEFER_NORM and pending is not None:
                    emit_normalize(*pending)
                    pending = None
                for h in range(HL):
                    nc.tensor.matmul(
                        o_tiles[h][:, 0:512],
                        vaug[b][:, h, mt, :],
                        p_tiles[h][:, 0:512],
                        start=(mt == 0), stop=(mt == NMT - 1),
                    )
                for h in range(HL):
                    nc.tensor.matmul(
                        o_tiles[h][:, 512:1024],
                        vaug[b][:, h, mt, :],
                        p_tiles[h][:, 512:1024],
                        start=(mt == 0), stop=(mt == NMT - 1),
                    )
            if DEFER_NORM:
                pending = (b, q0, o_tiles)
            else:
                emit_normalize(b, q0, o_tiles)
        if pending is not None:
            emit_normalize(*pending)
            pending = None
        emit_proj(b, 0)
        emit_proj(b, NQC)


def _get_nc():
    key = (USE_F32R, INTERLEAVE_B, DEFER_NORM)
    if key not in _CACHE:
        _CACHE[key] = _build_nc()
    return _CACHE[key]


def kernel(x, w_qkv, w_proj, b_proj):
    x = np.asarray(x, dtype=np.float32)
    w_qkv = np.asarray(w_qkv, dtype=np.float32)
    w_proj = np.asarray(w_proj, dtype=np.float32)
    b_proj = np.asarray(b_proj, dtype=np.float32)

    x_flat = np.ascontiguousarray(x.reshape(BN, C))
    scale = np.float32(D ** -0.5)

    in_maps = []
    for r in range(N_CORES):
        h0 = r * HL
        cols = slice(h0 * D, h0 * D + CL)
        w_loc = np.concatenate(
            [w_qkv[:, 0 * C:1 * C][:, cols] * scale,
             w_qkv[:, 1 * C:2 * C][:, cols],
             w_qkv[:, 2 * C:3 * C][:, cols]], axis=1)
        wp_loc = w_proj[h0 * D:h0 * D + CL, :]
        in_maps.append({
            "x": x_flat,
            "w": np.ascontiguousarray(w_loc),
            "wp": np.ascontiguousarray(wp_loc),
        })

    nc = _get_nc()
    last_exc = None
    for _ in range(3):
        try:
            res = run_bass_kernel_spmd(
                nc, in_maps, core_ids=list(range(N_CORES)),
                trace=PROFILE, **({"trace_cores": [0]} if PROFILE else {}),
            )
            break
        except Exception as e:
            last_exc = e
    else:
        raise last_exc
    kernel.last_result = res

    y = res.results[0]["y"].astype(np.float64)
    for r in range(1, N_CORES):
        y += res.results[r]["y"]
    y = (y + b_proj).astype(np.float32)
    return y.reshape(B, N, C)

